# revision 6
# baseline (speedup 1.0000x reference)
"""Trainium2 Bass kernel for a 2-layer GraphSAGE (mean aggregation) GNN.

Contract: kernel(**inputs) takes the FULL inputs from setup_inputs() and
returns the FULL [50000, 128] float32 output, distributing work across 8
NeuronCores internally.

Strategy (self-contained; constants hardcoded for N=50000, E=600000, F=128):
  - Shard nodes (and their incoming edges) by dst range: core c owns nodes
    [c*6250, (c+1)*6250).
  - The whole datapath runs in bf16 (tolerance is 2e-2; measured end-to-end
    error of the bf16 pipeline is ~4e-3). The hard floor is the dma_gather
    descriptor rate (~3 ns/row gathered); all compute hides under it.
  - Per core, group edges by 128-wide dst blocks; within a block split by
    src gather table (layer 1: two x halves, since dma_gather indices are
    int16; layer 2: four h quarters, see below); pad each (block, table)
    edge list to a multiple of 128 (chunk) with dummy edges (idx 0,
    dstloc -1).
  - Gather bf16 rows (256B) from HBM with gpsimd.dma_gather in 1024-idx
    calls rotated over all 4 SWDGE queues.
  - Aggregation in transposed orientation: for each 128-edge chunk,
    onehot[e, v] = (dstloc[e] == v) on DVE; psum_aggT[f, v] += msgs^T via
    matmul(lhsT=msgs_chunk, rhs=onehot). The psum->SBUF eviction applies
    the mean via one scalar_tensor_tensor against a host-precomputed
    [128, v] broadcast of 1/max(deg,1). This directly yields agg^T
    (features on partitions) - no transpose, no on-device degree math.
  - hT_blk = relu(W_l^T aggT + W_r^T xT_blk + b): the stored [F, H] weights
    ARE lhsT for this orientation; xT (self features) is host-transposed and
    SBUF-resident. Layer 1 writes hT into a resident SBUF tile (reused as
    layer 2's self term) and PE-transposes each block to node-major for the
    bf16 h gather table in DRAM. Layer 2 writes transposed f32 output blocks
    straight to DRAM; the host undoes the transpose.
  - The h shard is AllGathered between layers in FOUR row-range quarters so
    each quarter's collective launches as soon as its 12-13 blocks finish,
    pipelining the collectives under layer 1's remaining compute; layer 2's
    later-quarter gathers ride their own SWDGE queue (table t -> queue t) so
    a quarter still in flight doesn't head-of-line-stall earlier quarters'
    gathers. Collectives run ~2.5x faster with wide rows, so the h tensors
    are declared f32 [rows/8, 512] (2KB rows) and viewed via reshape/bitcast
    as [rows, 128] bf16 for the gather table and block writes (bf16
    collectives also hit a ~1.5x slow path, another reason for the f32 view).
"""
import sys

sys.path.insert(0, "/opt/trn_rl_repo")

from contextlib import ExitStack

import numpy as np

N = 50000
E = 600000
F = 128
NC = 8
NPC = N // NC          # 6250 nodes per core
NB = (NPC + 127) // 128  # 49 dst blocks per core
NPCP = NB * 128        # 6272 padded nodes per core
NP = NC * NPCP         # 50176 padded total
TAB1 = N // 2          # 25000: layer-1 lo/hi table split
SBS = 4                # blocks per gather superbatch (= agg psum bufs)
QBLK = (12, 12, 12, 13)  # L1 blocks per AllGather quarter
QSTART = (0, 12, 24, 36)
QROWS = tuple(b * 128 for b in QBLK)  # (1536, 1536, 1536, 1664)
GMAX = 1024            # idxs per dma_gather call

_cache = {}


def _ceil_div(a, b):
    return -(-a // b)


def _host_prep(x, edge_index):
    """Build per-core padded gather/index/dstloc arrays (index bookkeeping)."""
    import ml_dtypes

    bf16 = ml_dtypes.bfloat16
    src = np.asarray(edge_index[0], dtype=np.int64)
    dst = np.asarray(edge_index[1], dtype=np.int64)
    core = dst // NPC
    blk = (dst % NPC) >> 7
    dloc = (dst % NPC) & 127

    def wrap(a):
        n = a.shape[1]
        w = np.ascontiguousarray(a.reshape(NC, n // 16, 16).transpose(0, 2, 1))
        return np.tile(w, (1, 8, 1))  # [NC, 128, n//16]

    def prep_layer(tab, rowid, T):
        """tab[e] in [0,T): which gather table; rowid[e]: row within table."""
        key = (core * NB + blk) * T + tab
        order = np.lexsort((rowid, key))
        s_row = rowid[order]
        s_dloc = dloc[order]
        s_key = key[order]
        bounds = np.searchsorted(s_key, np.arange(NC * NB * T + 1))
        cnt = (bounds[1:] - bounds[:-1]).reshape(NC, NB, T)
        chunks = _ceil_div(cnt, 128)
        ct = chunks.max(axis=0)          # [NB, T] chunks per (block, table)
        nch = ct.sum(axis=1)             # [NB]
        K = ct.sum(axis=0) * 128         # [T] padded idx count per table
        NCH = int(nch.sum())
        idx = [np.zeros((NC, int(K[t])), np.int16) for t in range(T)]
        dcols = np.full((NC, NCH * 128), -1.0, np.float32)
        for c in range(NC):
            off = [0] * T
            off_q = 0
            for b in range(NB):
                qb = 0
                for t in range(T):
                    i0 = bounds[(c * NB + b) * T + t]
                    i1 = bounds[(c * NB + b) * T + t + 1]
                    n = i1 - i0
                    idx[t][c, off[t]:off[t] + n] = s_row[i0:i1]
                    dcols[c, off_q + qb:off_q + qb + n] = s_dloc[i0:i1]
                    off[t] += int(ct[b, t]) * 128
                    qb += int(ct[b, t]) * 128
                off_q += int(nch[b]) * 128
        dl = np.ascontiguousarray(
            dcols.reshape(NC, NCH, 128).transpose(0, 2, 1))
        return dict(ct=tuple(tuple(int(v) for v in row) for row in ct),
                    idx=[wrap(a) for a in idx], dl=dl,
                    K=tuple(int(v) for v in K), NCH=NCH, T=T)

    # layer 1: split x table at row TAB1
    t1 = (src >= TAB1).astype(np.int64)
    r1 = np.where(t1 == 0, src, src - TAB1)
    L1 = prep_layer(t1, r1, 2)
    # layer 2: h is all-gathered in four quarters by within-core row ranges
    rc = src % NPC
    cc = src // NPC
    t2 = np.searchsorted(np.array(QSTART[1:]) * 128, rc, side="right")
    r2 = cc * np.array(QROWS)[t2] + (rc - np.array(QSTART)[t2] * 128)
    L2 = prep_layer(t2, r2, 4)

    # per-core 1/max(deg,1), broadcast across partitions (one [128, v] tile
    # per dst block, applied on the psum->SBUF eviction as the mean scaling)
    deg = np.zeros((NC, NPCP), np.float32)
    for c in range(NC):
        deg[c, :NPC] = np.bincount(dst[core == c] % NPC, minlength=NPC)
    recip = (1.0 / np.maximum(deg, 1.0)).astype(bf16)
    recip_bc = np.ascontiguousarray(
        np.broadcast_to(recip[:, None, :], (NC, 128, NPCP)))

    return {
        "L1": L1, "L2": L2, "recip_bc": recip_bc,
        "dstloc": np.concatenate([L1["dl"], L2["dl"]], axis=2),
        "NCH": L1["NCH"] + L2["NCH"],
    }


def _build(L1, L2, NCH, loop_reps=0, stage="full"):
    from concourse import bacc, tile
    from concourse.bass import mybir

    f32 = mybir.dt.float32
    bf16 = mybir.dt.bfloat16
    i16 = mybir.dt.int16
    AF = mybir.ActivationFunctionType
    OP = mybir.AluOpType

    nc = bacc.Bacc("TRN2", target_bir_lowering=False, debug=False, num_devices=NC,
                   num_swdge_queues=4)

    x_tab = nc.declare_dram_parameter("x_tab", [N, F], bf16, isOutput=False)
    xT_self = nc.declare_dram_parameter("xT_self", [128, NPCP], bf16, isOutput=False)
    d_idx = {}
    for t in range(2):
        d_idx[(1, t)] = nc.declare_dram_parameter(
            f"idx1_{t}", [128, L1["K"][t] // 16], i16, isOutput=False)
    for t in range(4):
        d_idx[(2, t)] = nc.declare_dram_parameter(
            f"idx2_{t}", [128, L2["K"][t] // 16], i16, isOutput=False)
    d_dstloc = nc.declare_dram_parameter("dstloc", [128, NCH], f32, isOutput=False)
    d_recip = nc.declare_dram_parameter("recip_bc", [128, NPCP], bf16, isOutput=False)
    d_wl1 = nc.declare_dram_parameter("wl1", [F, F], bf16, isOutput=False)
    d_wr1 = nc.declare_dram_parameter("wr1", [F, F], bf16, isOutput=False)
    d_wl2 = nc.declare_dram_parameter("wl2", [F, F], bf16, isOutput=False)
    d_wr2 = nc.declare_dram_parameter("wr2", [F, F], bf16, isOutput=False)
    d_b1 = nc.declare_dram_parameter("b1c", [128, 1], f32, isOutput=False)
    d_b2 = nc.declare_dram_parameter("b2c", [128, 1], f32, isOutput=False)
    d_iota = nc.declare_dram_parameter("iota", [128, 128], bf16, isOutput=False)
    d_ident = nc.declare_dram_parameter("ident", [128, 128], bf16, isOutput=False)
    d_outT = nc.declare_dram_parameter("outT", [128, NPCP], f32, isOutput=True)

    # h shard in four quarters so each AllGather launches as soon as its
    # blocks finish. Declared f32 with 2KB rows (collectives are ~2.5x
    # faster with wide rows and bf16 collectives hit a slow path); viewed
    # as [rows, 128] bf16 via reshape/bitcast at the write/gather sides.
    h_shard = [nc.dram_tensor(f"h_shard_{t}", [QROWS[t] // 8, 512], f32)
               for t in range(4)]
    h_full = [nc.dram_tensor(f"h_full_{t}", [NC * QROWS[t] // 8, 512], f32,
                             addr_space="Shared")
              for t in range(4)]
    h_shard_v = [h_shard[t].bitcast(bf16).reshape([QROWS[t], F]) for t in range(4)]
    h_full_v = [h_full[t].bitcast(bf16).reshape([NC * QROWS[t], F]) for t in range(4)]

    with tile.TileContext(nc) as tc, ExitStack() as ctx:
        pstat = ctx.enter_context(tc.tile_pool(name="stat", bufs=1))
        pg = ctx.enter_context(tc.tile_pool(name="pg", bufs=2))
        pone = ctx.enter_context(tc.tile_pool(name="pone", bufs=6))
        psm = ctx.enter_context(tc.tile_pool(name="psm", bufs=3))
        pnode = ctx.enter_context(tc.tile_pool(name="pnode", bufs=3))
        pps_agg = ctx.enter_context(tc.tile_pool(name="ppsagg", bufs=4, space="PSUM"))
        pps_t = ctx.enter_context(tc.tile_pool(name="ppst", bufs=2, space="PSUM"))
        pps_h = ctx.enter_context(tc.tile_pool(name="ppsh", bufs=2, space="PSUM"))

        iota_s = pstat.tile([128, 128], bf16, tag="iota")
        nc.sync.dma_start(out=iota_s[:], in_=d_iota[:])
        ident_s = pstat.tile([128, 128], bf16, tag="ident")
        nc.sync.dma_start(out=ident_s[:], in_=d_ident[:])
        wl1_s = pstat.tile([128, 128], bf16, tag="wl1")
        nc.sync.dma_start(out=wl1_s[:], in_=d_wl1[:])
        wr1_s = pstat.tile([128, 128], bf16, tag="wr1")
        nc.sync.dma_start(out=wr1_s[:], in_=d_wr1[:])
        wl2_s = pstat.tile([128, 128], bf16, tag="wl2")
        nc.sync.dma_start(out=wl2_s[:], in_=d_wl2[:])
        wr2_s = pstat.tile([128, 128], bf16, tag="wr2")
        nc.sync.dma_start(out=wr2_s[:], in_=d_wr2[:])
        b1_s = pstat.tile([128, 1], f32, tag="b1")
        nc.sync.dma_start(out=b1_s[:], in_=d_b1[:])
        b2_s = pstat.tile([128, 1], f32, tag="b2")
        nc.sync.dma_start(out=b2_s[:], in_=d_b2[:])
        dstloc_s = pstat.tile([128, NCH], f32, tag="dstloc")
        nc.sync.dma_start(out=dstloc_s[:], in_=d_dstloc[:])
        recip_s = pstat.tile([128, NPCP], bf16, tag="recip")
        nc.sync.dma_start(out=recip_s[:], in_=d_recip[:])
        xT_all = pstat.tile([128, NPCP], bf16, tag="xT")
        nc.sync.dma_start(out=xT_all[:], in_=xT_self[:])
        hT_all = pstat.tile([128, NPCP], bf16, tag="hT")
        idx_s = {}
        for (layer, t), d_t in d_idx.items():
            ts_ = pstat.tile([128, d_t.shape[1]], i16, tag=f"idx{layer}_{t}")
            nc.sync.dma_start(out=ts_[:], in_=d_t[:])
            idx_s[(layer, t)] = ts_

        self_qn = [0]

        def emit_ag(t):
            nc.gpsimd.collective_compute(
                "AllGather", OP.bypass, replica_groups=[list(range(NC))],
                ins=[h_shard[t][:]], outs=[h_full[t][:]])

        def emit_body(do_ag=True):
            layers = (1, 2)
            if stage == "gather1":
                layers = (1,)
            elif stage == "gather2":
                layers = (2,)
            for layer in layers:
                if layer == 1:
                    LP = L1
                    tabs = [x_tab[0:TAB1, :], x_tab[TAB1:N, :]]
                    wl_s, wr_s, bias_s = wl1_s, wr1_s, b1_s
                    q = 0
                else:
                    LP = L2
                    tabs = [h_full_v[t][:] for t in range(4)]
                    wl_s, wr_s, bias_s = wl2_s, wr2_s, b2_s
                    q = L1["NCH"]
                T = LP["T"]
                ct = LP["ct"]
                nch = [sum(ct[b]) for b in range(NB)]
                sbs = [list(range(s, min(s + SBS, NB))) for s in range(0, NB, SBS)]

                off = [0] * T
                ag_done = 0
                for sb in sbs:
                    skip_gather = stage == "aggonly"
                    g_t = []
                    for t in range(T):
                        nt = sum(ct[b][t] for b in sb) * 128
                        g = pg.tile([128, max(nt, 128) // 128, 128], bf16,
                                    tag=f"g{t}")
                        g_t.append(g)
                        for o in [] if skip_gather else range(0, nt, GMAX):
                            nn = min(GMAX, nt - o)
                            if layer == 1:
                                self_qn[0] += 1
                                qn = self_qn[0] % 4
                            else:
                                # table t rides queue t so a quarter whose
                                # AllGather is still in flight doesn't stall
                                # other quarters' gathers
                                qn = t
                            nc.gpsimd.dma_gather(
                                out_ap=g[:, o // 128:(o + nn) // 128, :],
                                in_ap=tabs[t],
                                idxs_ap=idx_s[(layer, t)][
                                    :, (off[t] + o) // 16:(off[t] + o + nn) // 16],
                                num_idxs=nn, num_idxs_reg=nn, elem_size=F,
                                single_packet=False, queue_num=qn)
                        off[t] += nt

                    if stage in ("gather", "gather1", "gather2"):
                        q += sum(nch[b] for b in sb)
                        continue
                    cols = [0] * T
                    ps_blocks = {}
                    for b in sb:
                        # all SBS blocks' aggregation matmuls are emitted
                        # before any tail so the in-order PE queue doesn't
                        # stall the next block's aggregation behind ACT
                        # tail work
                        ps_aggT = pps_agg.tile([128, 128], f32, tag="psagg")
                        ps_blocks[b] = ps_aggT
                        j = 0
                        for t in range(T):
                            for _ in range(ct[b][t]):
                                onehot = pone.tile([128, 128], bf16, tag="onehot")
                                nc.vector.tensor_scalar(
                                    onehot[:], iota_s[:], dstloc_s[:, q:q + 1],
                                    None, OP.is_equal)
                                nc.tensor.matmul(
                                    ps_aggT[:], g_t[t][:, cols[t], :], onehot[:],
                                    start=(j == 0), stop=(j == nch[b] - 1))
                                cols[t] += 1
                                q += 1
                                j += 1
                    if stage == "agg":
                        continue
                    for b in sb:
                        ps_aggT = ps_blocks[b]
                        aggTs = psm.tile([128, 128], bf16, tag="aggTs")
                        nc.vector.scalar_tensor_tensor(
                            aggTs[:], ps_aggT[:], 1.0,
                            recip_s[:, b * 128:(b + 1) * 128], OP.mult, OP.mult)
                        rhs2 = (xT_all if layer == 1 else hT_all)[:, b * 128:(b + 1) * 128]
                        ps_h = pps_h.tile([128, 128], f32, tag="psh")
                        nc.tensor.matmul(ps_h[:], wl_s[:], aggTs[:], start=True, stop=False)
                        nc.tensor.matmul(ps_h[:], wr_s[:], rhs2, start=False, stop=True)
                        if layer == 1:
                            hT_blk = hT_all[:, b * 128:(b + 1) * 128]
                            nc.scalar.activation(hT_blk, ps_h[:], AF.Relu, bias=b1_s[:])
                            ps_t = pps_t.tile([128, 128], bf16, tag="pst")
                            nc.tensor.transpose(ps_t[:], hT_blk, ident_s[:])
                            nodeb = pnode.tile([128, 128], bf16, tag="nodeb")
                            nc.scalar.copy(nodeb[:], ps_t[:])
                            qt = 0 if b < 12 else (1 if b < 24 else (2 if b < 36 else 3))
                            lb = b - QSTART[qt]
                            nc.sync.dma_start(
                                out=h_shard_v[qt][lb * 128:(lb + 1) * 128, :],
                                in_=nodeb[:])
                        else:
                            outb = pnode.tile([128, 128], f32, tag="outb")
                            nc.scalar.activation(outb[:], ps_h[:], AF.Relu, bias=b2_s[:])
                            nc.sync.dma_start(
                                out=d_outT[:, b * 128:(b + 1) * 128], in_=outb[:])
                    # launch each quarter's AllGather as soon as its last
                    # block's h rows are written
                    if layer == 1 and do_ag and stage == "full":
                        while ag_done < 4 and sb[-1] >= QSTART[ag_done] + QBLK[ag_done] - 1:
                            emit_ag(ag_done)
                            ag_done += 1

        if loop_reps:
            # timing variant: collectives can't live inside control flow;
            # init the h_full quarters once and loop the 2-layer pipeline
            for t in range(4):
                emit_ag(t)
            with tc.For_i(0, loop_reps, 1):
                emit_body(do_ag=False)
        else:
            emit_body(do_ag=True)
    nc.compile()
    return nc


def _get_program(prep, loop_reps=0, stage="full"):
    key = (prep["L1"]["ct"], prep["L2"]["ct"], loop_reps, stage)
    if key not in _cache:
        _cache[key] = _build(prep["L1"], prep["L2"], prep["NCH"], loop_reps, stage)
    return _cache[key]


def _in_maps(prep, x, W1_l, b1, W1_r, W2_l, b2, W2_r):
    import ml_dtypes

    bf16 = ml_dtypes.bfloat16
    x_bf = np.ascontiguousarray(np.asarray(x, np.float32)).astype(bf16)
    iota = np.ascontiguousarray(
        np.broadcast_to(np.arange(128, dtype=np.float32), (128, 128))).astype(bf16)
    ident = np.eye(128, dtype=np.float32).astype(bf16)
    common = {
        "x_tab": x_bf,
        "wl1": np.ascontiguousarray(np.asarray(W1_l, np.float32)).astype(bf16),
        "wr1": np.ascontiguousarray(np.asarray(W1_r, np.float32)).astype(bf16),
        "wl2": np.ascontiguousarray(np.asarray(W2_l, np.float32)).astype(bf16),
        "wr2": np.ascontiguousarray(np.asarray(W2_r, np.float32)).astype(bf16),
        "b1c": np.ascontiguousarray(np.asarray(b1, np.float32).reshape(128, 1)),
        "b2c": np.ascontiguousarray(np.asarray(b2, np.float32).reshape(128, 1)),
        "iota": iota,
        "ident": ident,
    }
    maps = []
    for c in range(NC):
        xTs = np.zeros((128, NPCP), np.float32)
        xTs[:, :NPC] = np.asarray(x, np.float32)[c * NPC:(c + 1) * NPC].T
        m = dict(common)
        m["xT_self"] = np.ascontiguousarray(xTs).astype(bf16)
        for t in range(2):
            m[f"idx1_{t}"] = np.ascontiguousarray(prep["L1"]["idx"][t][c])
        for t in range(4):
            m[f"idx2_{t}"] = np.ascontiguousarray(prep["L2"]["idx"][t][c])
        m["dstloc"] = np.ascontiguousarray(prep["dstloc"][c])
        m["recip_bc"] = np.ascontiguousarray(prep["recip_bc"][c])
        maps.append(m)
    return maps


def kernel(x, edge_index, W1_l, b1, W1_r, W2_l, b2, W2_r):
    from concourse.bass_utils import run_bass_kernel_spmd

    x = np.asarray(x, np.float32)
    assert x.shape == (N, F) and np.asarray(edge_index).shape == (2, E)
    prep = _host_prep(x, edge_index)
    nc = _get_program(prep)
    maps = _in_maps(prep, x, W1_l, b1, W1_r, W2_l, b2, W2_r)
    res = run_bass_kernel_spmd(nc, maps, list(range(NC)))
    out = np.concatenate(
        [np.asarray(res.results[c]["outT"], np.float32).T[:NPC] for c in range(NC)],
        axis=0)
    return out.astype(np.float32)


# revision 7
# speedup vs baseline: 1.9773x; 1.9773x over previous
"""Trainium2 Bass kernel for a 2-layer GraphSAGE (mean aggregation) GNN.

Contract: kernel(**inputs) takes the FULL inputs from setup_inputs() and
returns the FULL [50000, 128] float32 output, distributing work across 8
NeuronCores internally.

Strategy (self-contained; constants hardcoded for N=50000, E=600000, F=128):
  - Shard nodes (and their incoming edges) by dst range: core c owns nodes
    [c*6250, (c+1)*6250).
  - The whole datapath runs in bf16 (tolerance is 2e-2; measured end-to-end
    error of the bf16 pipeline is ~4e-3). The hard floor is the dma_gather
    descriptor rate (~2.9 ns/row gathered); all compute hides under it.
  - Per core, group edges by 128-wide dst blocks; within a block split by
    src table half (dma_gather indices are int16, so each gather table is
    split into two <32768-row halves); pad each (block, table) edge list to
    a multiple of 128 (chunk) with dummy edges (idx 0, dstloc -1).
  - Gather bf16 x[src] rows (256B each) from HBM with gpsimd.dma_gather in
    1024-idx calls rotated over all 4 SWDGE queues.
  - Aggregation in transposed orientation: for each 128-edge chunk,
    onehot_scaled[e, v] = (dstloc[e] == v) * recip_deg[v] built in ONE DVE
    scalar_tensor_tensor op (is_equal then mult with a host-precomputed
    [128, v] broadcast of 1/max(deg,1)); psum_aggT[f, v] += msgs^T via
    matmul(lhsT=msgs_chunk, rhs=onehot_scaled). This directly yields agg^T
    (features on partitions) including the mean scaling - no transpose, no
    on-device degree math.
  - hT_blk = relu(W_l^T aggT + W_r^T xT_blk + b): the stored [F, H] weights
    ARE lhsT for this orientation; xT (self features) is host-transposed and
    SBUF-resident. Layer 1 writes hT into a resident SBUF tile (reused as
    layer 2's self term) and PE-transposes each block to node-major for the
    bf16 h gather table in DRAM. Layer 2 writes transposed f32 output blocks
    straight to DRAM; the host undoes the transpose.
  - The h shard is AllGathered between layers in two row-range halves
    (separate internal DRAM tensors): the first AllGather's inputs are ready
    while layer 1's second half still computes, and layer 2's a-side gathers
    overlap the second AllGather. Layer 2 repeats the pipeline on h with its
    own (block, table) edge grouping keyed to the two gathered halves
    (a-side gathers ride queues {0,1}, b-side {2,3} so b-gathers blocked on
    the second AllGather don't head-of-line-stall a-gathers).
"""
import sys

sys.path.insert(0, "/opt/trn_rl_repo")

from contextlib import ExitStack

import numpy as np

N = 50000
E = 600000
F = 128
NC = 8
NPC = N // NC          # 6250 nodes per core
NB = (NPC + 127) // 128  # 49 dst blocks per core
NPCP = NB * 128        # 6272 padded nodes per core
NP = NC * NPCP         # 50176 padded total
TAB1 = N // 2          # 25000: layer-1 lo/hi table split
SBS = 4                # blocks per gather superbatch (= agg psum bufs)
NBA = 25               # L1 blocks whose h rows go in the first AllGather
HA_ROWS = NBA * 128    # 3200 rows/core in h_shard_a
HB_ROWS = (NB - NBA) * 128  # 3072 rows/core in h_shard_b
GMAX = 1024            # idxs per dma_gather call

_cache = {}


def _ceil_div(a, b):
    return -(-a // b)


def _host_prep(x, edge_index):
    """Build per-core padded gather/index/dstloc arrays (index bookkeeping)."""
    import ml_dtypes

    bf16 = ml_dtypes.bfloat16
    src = np.asarray(edge_index[0], dtype=np.int64)
    dst = np.asarray(edge_index[1], dtype=np.int64)
    core = dst // NPC
    blk = (dst % NPC) >> 7
    dloc = (dst % NPC) & 127

    def wrap(a):
        n = a.shape[1]
        w = np.ascontiguousarray(a.reshape(NC, n // 16, 16).transpose(0, 2, 1))
        return np.tile(w, (1, 8, 1))  # [NC, 128, n//16]

    def prep_layer(tab, rowid):
        """tab[e] in {0,1}: which gather table; rowid[e]: row within table."""
        key = (core * NB + blk) * 2 + tab
        order = np.lexsort((rowid, key))
        s_row = rowid[order]
        s_dloc = dloc[order]
        s_key = key[order]
        bounds = np.searchsorted(s_key, np.arange(NC * NB * 2 + 1))
        cnt = (bounds[1:] - bounds[:-1]).reshape(NC, NB, 2)
        chunks = _ceil_div(cnt, 128)
        c_lo = chunks[:, :, 0].max(axis=0)
        c_hi = chunks[:, :, 1].max(axis=0)
        nch = c_lo + c_hi
        K_lo = int(c_lo.sum()) * 128
        K_hi = int(c_hi.sum()) * 128
        NCH = int(nch.sum())
        idx_lo = np.zeros((NC, K_lo), np.int16)
        idx_hi = np.zeros((NC, K_hi), np.int16)
        dcols = np.full((NC, NCH * 128), -1.0, np.float32)
        for c in range(NC):
            off_lo = off_hi = off_q = 0
            for b in range(NB):
                i0 = bounds[(c * NB + b) * 2]
                i1 = bounds[(c * NB + b) * 2 + 1]
                i2 = bounds[(c * NB + b) * 2 + 2]
                nlo = i1 - i0
                nhi = i2 - i1
                idx_lo[c, off_lo:off_lo + nlo] = s_row[i0:i1]
                idx_hi[c, off_hi:off_hi + nhi] = s_row[i1:i2]
                dcols[c, off_q:off_q + nlo] = s_dloc[i0:i1]
                dcols[c, off_q + c_lo[b] * 128:off_q + c_lo[b] * 128 + nhi] = s_dloc[i1:i2]
                off_lo += c_lo[b] * 128
                off_hi += c_hi[b] * 128
                off_q += nch[b] * 128
        dl = np.ascontiguousarray(
            dcols.reshape(NC, NCH, 128).transpose(0, 2, 1))
        return dict(c_lo=tuple(int(v) for v in c_lo), c_hi=tuple(int(v) for v in c_hi),
                    idx_lo=wrap(idx_lo), idx_hi=wrap(idx_hi), dl=dl,
                    K_lo=K_lo, K_hi=K_hi, NCH=NCH)

    # layer 1: split x table at row TAB1
    t1 = (src >= TAB1).astype(np.int64)
    r1 = np.where(t1 == 0, src, src - TAB1)
    L1 = prep_layer(t1, r1)
    # layer 2: h is all-gathered in two halves by within-core row ranges
    # (rows < HA_ROWS go to h_full_a, rest to h_full_b)
    rc = src % NPC
    cc = src // NPC
    t2 = (rc >= HA_ROWS).astype(np.int64)
    r2 = np.where(t2 == 0, cc * HA_ROWS + rc, cc * HB_ROWS + (rc - HA_ROWS))
    L2 = prep_layer(t2, r2)

    # per-core 1/max(deg,1), broadcast across partitions (one [128, v] tile
    # per dst block, fused into the onehot build as the mean scaling)
    deg = np.zeros((NC, NPCP), np.float32)
    for c in range(NC):
        deg[c, :NPC] = np.bincount(dst[core == c] % NPC, minlength=NPC)
    recip = (1.0 / np.maximum(deg, 1.0)).astype(bf16)
    recip_bc = np.ascontiguousarray(
        np.broadcast_to(recip[:, None, :], (NC, 128, NPCP)))

    return {
        "L1": L1, "L2": L2, "recip_bc": recip_bc,
        "dstloc": np.concatenate([L1["dl"], L2["dl"]], axis=2),
        "NCH": L1["NCH"] + L2["NCH"],
    }


def _build(L1, L2, NCH, loop_reps=0, stage="full"):
    from concourse import bacc, tile
    from concourse.bass import mybir

    f32 = mybir.dt.float32
    bf16 = mybir.dt.bfloat16
    i16 = mybir.dt.int16
    AF = mybir.ActivationFunctionType
    OP = mybir.AluOpType

    nc = bacc.Bacc("TRN2", target_bir_lowering=False, debug=False, num_devices=NC,
                   num_swdge_queues=4)

    x_tab = nc.declare_dram_parameter("x_tab", [N, F], bf16, isOutput=False)
    xT_self = nc.declare_dram_parameter("xT_self", [128, NPCP], bf16, isOutput=False)
    d_idx1_lo = nc.declare_dram_parameter("idx1_lo", [128, L1["K_lo"] // 16], i16, isOutput=False)
    d_idx1_hi = nc.declare_dram_parameter("idx1_hi", [128, L1["K_hi"] // 16], i16, isOutput=False)
    d_idx2_lo = nc.declare_dram_parameter("idx2_lo", [128, L2["K_lo"] // 16], i16, isOutput=False)
    d_idx2_hi = nc.declare_dram_parameter("idx2_hi", [128, L2["K_hi"] // 16], i16, isOutput=False)
    d_dstloc = nc.declare_dram_parameter("dstloc", [128, NCH], f32, isOutput=False)
    d_recip = nc.declare_dram_parameter("recip_bc", [128, NPCP], bf16, isOutput=False)
    d_wl1 = nc.declare_dram_parameter("wl1", [F, F], bf16, isOutput=False)
    d_wr1 = nc.declare_dram_parameter("wr1", [F, F], bf16, isOutput=False)
    d_wl2 = nc.declare_dram_parameter("wl2", [F, F], bf16, isOutput=False)
    d_wr2 = nc.declare_dram_parameter("wr2", [F, F], bf16, isOutput=False)
    d_b1 = nc.declare_dram_parameter("b1c", [128, 1], f32, isOutput=False)
    d_b2 = nc.declare_dram_parameter("b2c", [128, 1], f32, isOutput=False)
    d_iota = nc.declare_dram_parameter("iota", [128, 128], bf16, isOutput=False)
    d_ident = nc.declare_dram_parameter("ident", [128, 128], bf16, isOutput=False)
    d_outT = nc.declare_dram_parameter("outT", [128, NPCP], f32, isOutput=True)

    # h shard in two pieces so the first AllGather can launch while the
    # second half of layer 1 is still computing, and layer 2's a-side
    # gathers can proceed while the second AllGather is in flight.
    # bf16 payload declared as f32 with wide 2KB rows: bf16 collectives hit
    # a slow path (~1.5x byte-equivalent f32) and collectives run ~2.5x
    # faster with wide rows. Viewed as [rows, 128] bf16 via bitcast/reshape
    # at the gather/store boundaries.
    h_shard_a = nc.dram_tensor("h_shard_a", [HA_ROWS // 8, 512], f32)
    h_shard_b = nc.dram_tensor("h_shard_b", [HB_ROWS // 8, 512], f32)
    h_full_a = nc.dram_tensor("h_full_a", [NC * HA_ROWS // 8, 512], f32, addr_space="Shared")
    h_full_b = nc.dram_tensor("h_full_b", [NC * HB_ROWS // 8, 512], f32, addr_space="Shared")
    h_shard_av = h_shard_a.bitcast(bf16).reshape([HA_ROWS, F])
    h_shard_bv = h_shard_b.bitcast(bf16).reshape([HB_ROWS, F])
    h_full_av = h_full_a.bitcast(bf16).reshape([NC * HA_ROWS, F])
    h_full_bv = h_full_b.bitcast(bf16).reshape([NC * HB_ROWS, F])

    with tile.TileContext(nc) as tc, ExitStack() as ctx:
        pstat = ctx.enter_context(tc.tile_pool(name="stat", bufs=1))
        pg = ctx.enter_context(tc.tile_pool(name="pg", bufs=2))
        pone = ctx.enter_context(tc.tile_pool(name="pone", bufs=6))
        psm = ctx.enter_context(tc.tile_pool(name="psm", bufs=3))
        pnode = ctx.enter_context(tc.tile_pool(name="pnode", bufs=3))
        pps_agg = ctx.enter_context(tc.tile_pool(name="ppsagg", bufs=4, space="PSUM"))
        pps_t = ctx.enter_context(tc.tile_pool(name="ppst", bufs=2, space="PSUM"))
        pps_h = ctx.enter_context(tc.tile_pool(name="ppsh", bufs=2, space="PSUM"))

        iota_s = pstat.tile([128, 128], bf16, tag="iota")
        nc.sync.dma_start(out=iota_s[:], in_=d_iota[:])
        ident_s = pstat.tile([128, 128], bf16, tag="ident")
        nc.sync.dma_start(out=ident_s[:], in_=d_ident[:])
        wl1_s = pstat.tile([128, 128], bf16, tag="wl1")
        nc.sync.dma_start(out=wl1_s[:], in_=d_wl1[:])
        wr1_s = pstat.tile([128, 128], bf16, tag="wr1")
        nc.sync.dma_start(out=wr1_s[:], in_=d_wr1[:])
        wl2_s = pstat.tile([128, 128], bf16, tag="wl2")
        nc.sync.dma_start(out=wl2_s[:], in_=d_wl2[:])
        wr2_s = pstat.tile([128, 128], bf16, tag="wr2")
        nc.sync.dma_start(out=wr2_s[:], in_=d_wr2[:])
        b1_s = pstat.tile([128, 1], f32, tag="b1")
        nc.sync.dma_start(out=b1_s[:], in_=d_b1[:])
        b2_s = pstat.tile([128, 1], f32, tag="b2")
        nc.sync.dma_start(out=b2_s[:], in_=d_b2[:])
        dstloc_s = pstat.tile([128, NCH], f32, tag="dstloc")
        nc.sync.dma_start(out=dstloc_s[:], in_=d_dstloc[:])
        recip_s = pstat.tile([128, NPCP], bf16, tag="recip")
        nc.sync.dma_start(out=recip_s[:], in_=d_recip[:])
        xT_all = pstat.tile([128, NPCP], bf16, tag="xT")
        nc.sync.dma_start(out=xT_all[:], in_=xT_self[:])
        hT_all = pstat.tile([128, NPCP], bf16, tag="hT")
        idx_s = {}
        for nm, d_t, wid in (("1lo", d_idx1_lo, L1["K_lo"]), ("1hi", d_idx1_hi, L1["K_hi"]),
                             ("2lo", d_idx2_lo, L2["K_lo"]), ("2hi", d_idx2_hi, L2["K_hi"])):
            t = pstat.tile([128, wid // 16], i16, tag=f"idx{nm}")
            nc.sync.dma_start(out=t[:], in_=d_t[:])
            idx_s[nm] = t

        self_qn = [0]

        def emit_body(do_ag=True):
            layers = (1, 2)
            if stage == "gather1":
                layers = (1,)
            elif stage == "gather2":
                layers = (2,)
            for layer in layers:
                if layer == 1:
                    LP = L1
                    lo_ap = x_tab[0:TAB1, :]
                    hi_ap = x_tab[TAB1:N, :]
                    t_lo, t_hi = idx_s["1lo"], idx_s["1hi"]
                    wl_s, wr_s, bias_s = wl1_s, wr1_s, b1_s
                    q = 0
                else:
                    LP = L2
                    lo_ap = h_full_av[:]
                    hi_ap = h_full_bv[:]
                    t_lo, t_hi = idx_s["2lo"], idx_s["2hi"]
                    wl_s, wr_s, bias_s = wl2_s, wr2_s, b2_s
                    q = L1["NCH"]
                c_lo, c_hi = LP["c_lo"], LP["c_hi"]
                nch = [c_lo[b] + c_hi[b] for b in range(NB)]
                sbs = [list(range(s, min(s + SBS, NB))) for s in range(0, NB, SBS)]

                off_lo = 0
                off_hi = 0
                for sb in sbs:
                    nlo = sum(c_lo[b] for b in sb) * 128
                    nhi = sum(c_hi[b] for b in sb) * 128
                    skip_gather = stage == "aggonly"

                    # layer 2: keep a-side (h_full_a) gathers on queues {0,1}
                    # and b-side on {2,3} so b-gathers blocked on the second
                    # AllGather don't head-of-line-stall later a-gathers
                    # whose data is already available.
                    def qpick(side):
                        self_qn[0] += 1
                        if layer == 1:
                            return self_qn[0] % 4
                        return (self_qn[0] % 2) + (0 if side == 0 else 2)
                    g_lo = pg.tile([128, nlo // 128, 128], bf16, tag="glo")
                    for o in [] if skip_gather else range(0, nlo, GMAX):
                        nn = min(GMAX, nlo - o)
                        nc.gpsimd.dma_gather(
                            out_ap=g_lo[:, o // 128:(o + nn) // 128, :], in_ap=lo_ap,
                            idxs_ap=t_lo[:, (off_lo + o) // 16:(off_lo + o + nn) // 16],
                            num_idxs=nn, num_idxs_reg=nn, elem_size=F,
                            single_packet=False, queue_num=qpick(0))
                    g_hi = pg.tile([128, nhi // 128, 128], bf16, tag="ghi")
                    for o in [] if skip_gather else range(0, nhi, GMAX):
                        nn = min(GMAX, nhi - o)
                        nc.gpsimd.dma_gather(
                            out_ap=g_hi[:, o // 128:(o + nn) // 128, :], in_ap=hi_ap,
                            idxs_ap=t_hi[:, (off_hi + o) // 16:(off_hi + o + nn) // 16],
                            num_idxs=nn, num_idxs_reg=nn, elem_size=F,
                            single_packet=False, queue_num=qpick(1))
                    off_lo += nlo
                    off_hi += nhi

                    if stage in ("gather", "gather1", "gather2"):
                        q += sum(nch[b] for b in sb)
                        continue
                    col_lo = 0
                    col_hi = 0
                    ps_blocks = {}
                    for b in sb:
                        # all SBS blocks' aggregation matmuls are emitted
                        # before any tail so the in-order PE queue doesn't
                        # stall the next block's aggregation behind ACT
                        # tail work
                        ps_aggT = pps_agg.tile([128, 128], f32, tag="psagg")
                        ps_blocks[b] = ps_aggT
                        for j in range(nch[b]):
                            onehot = pone.tile([128, 128], bf16, tag="onehot")
                            nc.vector.tensor_scalar(
                                onehot[:], iota_s[:], dstloc_s[:, q:q + 1], None,
                                OP.is_equal)
                            if j < c_lo[b]:
                                rhs = g_lo[:, col_lo, :]
                                col_lo += 1
                            else:
                                rhs = g_hi[:, col_hi, :]
                                col_hi += 1
                            nc.tensor.matmul(
                                ps_aggT[:], rhs, onehot[:],
                                start=(j == 0), stop=(j == nch[b] - 1))
                            q += 1
                    if stage == "agg":
                        continue
                    for b in sb:
                        ps_aggT = ps_blocks[b]
                        aggTs = psm.tile([128, 128], bf16, tag="aggTs")
                        nc.vector.scalar_tensor_tensor(
                            aggTs[:], ps_aggT[:], 1.0,
                            recip_s[:, b * 128:(b + 1) * 128], OP.mult, OP.mult)
                        rhs2 = (xT_all if layer == 1 else hT_all)[:, b * 128:(b + 1) * 128]
                        ps_h = pps_h.tile([128, 128], f32, tag="psh")
                        nc.tensor.matmul(ps_h[:], wl_s[:], aggTs[:], start=True, stop=False)
                        nc.tensor.matmul(ps_h[:], wr_s[:], rhs2, start=False, stop=True)
                        if layer == 1:
                            hT_blk = hT_all[:, b * 128:(b + 1) * 128]
                            nc.scalar.activation(hT_blk, ps_h[:], AF.Relu, bias=b1_s[:])
                            ps_t = pps_t.tile([128, 128], bf16, tag="pst")
                            nc.tensor.transpose(ps_t[:], hT_blk, ident_s[:])
                            nodeb = pnode.tile([128, 128], bf16, tag="nodeb")
                            nc.scalar.copy(nodeb[:], ps_t[:])
                            if b < NBA:
                                nc.sync.dma_start(
                                    out=h_shard_av[b * 128:(b + 1) * 128, :],
                                    in_=nodeb[:])
                            else:
                                nc.sync.dma_start(
                                    out=h_shard_bv[(b - NBA) * 128:(b - NBA + 1) * 128, :],
                                    in_=nodeb[:])
                        else:
                            outb = pnode.tile([128, 128], f32, tag="outb")
                            nc.scalar.activation(outb[:], ps_h[:], AF.Relu, bias=b2_s[:])
                            nc.sync.dma_start(
                                out=d_outT[:, b * 128:(b + 1) * 128], in_=outb[:])

                if layer == 1 and do_ag:
                    nc.gpsimd.collective_compute(
                        "AllGather", OP.bypass, replica_groups=[list(range(NC))],
                        ins=[h_shard_a[:]], outs=[h_full_a[:]])
                    nc.gpsimd.collective_compute(
                        "AllGather", OP.bypass, replica_groups=[list(range(NC))],
                        ins=[h_shard_b[:]], outs=[h_full_b[:]])

        if loop_reps:
            # timing variant: collectives can't live inside control flow;
            # init the h_full halves once and loop the 2-layer pipeline
            nc.gpsimd.collective_compute(
                "AllGather", OP.bypass, replica_groups=[list(range(NC))],
                ins=[h_shard_a[:]], outs=[h_full_a[:]])
            nc.gpsimd.collective_compute(
                "AllGather", OP.bypass, replica_groups=[list(range(NC))],
                ins=[h_shard_b[:]], outs=[h_full_b[:]])
            with tc.For_i(0, loop_reps, 1):
                emit_body(do_ag=False)
        else:
            emit_body(do_ag=True)
    nc.compile()
    return nc


def _get_program(prep, loop_reps=0, stage="full"):
    key = (prep["L1"]["c_lo"], prep["L1"]["c_hi"],
           prep["L2"]["c_lo"], prep["L2"]["c_hi"], loop_reps, stage)
    if key not in _cache:
        _cache[key] = _build(prep["L1"], prep["L2"], prep["NCH"], loop_reps, stage)
    return _cache[key]


def _in_maps(prep, x, W1_l, b1, W1_r, W2_l, b2, W2_r):
    import ml_dtypes

    bf16 = ml_dtypes.bfloat16
    x_bf = np.ascontiguousarray(np.asarray(x, np.float32)).astype(bf16)
    iota = np.ascontiguousarray(
        np.broadcast_to(np.arange(128, dtype=np.float32), (128, 128))).astype(bf16)
    ident = np.eye(128, dtype=np.float32).astype(bf16)
    common = {
        "x_tab": x_bf,
        "wl1": np.ascontiguousarray(np.asarray(W1_l, np.float32)).astype(bf16),
        "wr1": np.ascontiguousarray(np.asarray(W1_r, np.float32)).astype(bf16),
        "wl2": np.ascontiguousarray(np.asarray(W2_l, np.float32)).astype(bf16),
        "wr2": np.ascontiguousarray(np.asarray(W2_r, np.float32)).astype(bf16),
        "b1c": np.ascontiguousarray(np.asarray(b1, np.float32).reshape(128, 1)),
        "b2c": np.ascontiguousarray(np.asarray(b2, np.float32).reshape(128, 1)),
        "iota": iota,
        "ident": ident,
    }
    maps = []
    for c in range(NC):
        xTs = np.zeros((128, NPCP), np.float32)
        xTs[:, :NPC] = np.asarray(x, np.float32)[c * NPC:(c + 1) * NPC].T
        m = dict(common)
        m["xT_self"] = np.ascontiguousarray(xTs).astype(bf16)
        m["idx1_lo"] = np.ascontiguousarray(prep["L1"]["idx_lo"][c])
        m["idx1_hi"] = np.ascontiguousarray(prep["L1"]["idx_hi"][c])
        m["idx2_lo"] = np.ascontiguousarray(prep["L2"]["idx_lo"][c])
        m["idx2_hi"] = np.ascontiguousarray(prep["L2"]["idx_hi"][c])
        m["dstloc"] = np.ascontiguousarray(prep["dstloc"][c])
        m["recip_bc"] = np.ascontiguousarray(prep["recip_bc"][c])
        maps.append(m)
    return maps


def kernel(x, edge_index, W1_l, b1, W1_r, W2_l, b2, W2_r):
    from concourse.bass_utils import run_bass_kernel_spmd

    x = np.asarray(x, np.float32)
    assert x.shape == (N, F) and np.asarray(edge_index).shape == (2, E)
    prep = _host_prep(x, edge_index)
    nc = _get_program(prep)
    maps = _in_maps(prep, x, W1_l, b1, W1_r, W2_l, b2, W2_r)
    res = run_bass_kernel_spmd(nc, maps, list(range(NC)))
    out = np.concatenate(
        [np.asarray(res.results[c]["outT"], np.float32).T[:NPC] for c in range(NC)],
        axis=0)
    return out.astype(np.float32)


# revision 8
# speedup vs baseline: 2.0686x; 1.0462x over previous
"""Trainium2 Bass kernel for a 2-layer GraphSAGE (mean aggregation) GNN.

Contract: kernel(**inputs) takes the FULL inputs from setup_inputs() and
returns the FULL [50000, 128] float32 output, distributing work across 8
NeuronCores internally.

Strategy (self-contained; constants hardcoded for N=50000, E=600000, F=128):
  - Shard nodes (and their incoming edges) by dst range: core c owns nodes
    [c*6250, (c+1)*6250).
  - The whole datapath runs in bf16 (tolerance is 2e-2; measured end-to-end
    error of the bf16 pipeline is ~4e-3). The hard floor is the dma_gather
    descriptor rate (~2.9 ns/row gathered); all compute hides under it.
  - Per core, group edges by 128-wide dst blocks; within a block split by
    src table half (dma_gather indices are int16, so each gather table is
    split into two <32768-row halves); pad each (block, table) edge list to
    a multiple of 128 (chunk) with dummy edges (idx 0, dstloc -1).
  - Gather bf16 x[src] rows (256B each) from HBM with gpsimd.dma_gather in
    1024-idx calls rotated over all 4 SWDGE queues.
  - Aggregation in transposed orientation: for each 128-edge chunk,
    onehot[e, v] = (dstloc[e] == v) in one DVE tensor_scalar op;
    psum_aggT[f, v] += msgs^T via matmul(lhsT=msgs_chunk, rhs=onehot). The
    psum->SBUF eviction applies the mean via one scalar_tensor_tensor
    against a host-precomputed [128, v] broadcast of 1/max(deg,1). This
    directly yields agg^T (features on partitions) - no transpose, no
    on-device degree math.
  - hT_blk = relu(W_l^T aggT + W_r^T xT_blk + b): the stored [F, H] weights
    ARE lhsT for this orientation; xT (self features) is host-transposed and
    SBUF-resident. Layer 1 writes hT into a resident SBUF tile (reused as
    layer 2's self term) and PE-transposes each block to node-major for the
    bf16 h gather table in DRAM. Layer 2 writes transposed f32 output blocks
    straight to DRAM; the host undoes the transpose.
  - The h shard is AllGathered between layers in two row-range halves
    (separate internal DRAM tensors): the first AllGather's inputs are ready
    while layer 1's second half still computes, and layer 2's a-side gathers
    overlap the second AllGather. Layer 2 repeats the pipeline on h with its
    own (block, table) edge grouping keyed to the two gathered halves
    (a-side gathers ride queues {0,1}, b-side {2,3} so b-gathers blocked on
    the second AllGather don't head-of-line-stall a-gathers). The h tensors
    are declared f32 with wide 2KB rows and bitcast/reshaped to [rows, 128]
    bf16 at the write/gather boundaries: bf16 collectives hit a ~1.5x slow
    path and narrow-row collectives are ~2.5x slower than wide-row ones.
"""
import sys

sys.path.insert(0, "/opt/trn_rl_repo")

from contextlib import ExitStack

import numpy as np

N = 50000
E = 600000
F = 128
NC = 8
NPC = N // NC          # 6250 nodes per core
NB = (NPC + 127) // 128  # 49 dst blocks per core
NPCP = NB * 128        # 6272 padded nodes per core
NP = NC * NPCP         # 50176 padded total
TAB1 = N // 2          # 25000: layer-1 lo/hi table split
SBS = 4                # blocks per gather superbatch (= agg psum bufs)
NBA = 25               # L1 blocks whose h rows go in the first AllGather
HA_ROWS = NBA * 128    # 3200 rows/core in h_shard_a
HB_ROWS = (NB - NBA) * 128  # 3072 rows/core in h_shard_b
GMAX = 1024            # idxs per dma_gather call

_cache = {}


def _ceil_div(a, b):
    return -(-a // b)


def _host_prep(x, edge_index):
    """Build per-core padded gather/index/dstloc arrays (index bookkeeping)."""
    import ml_dtypes

    bf16 = ml_dtypes.bfloat16
    src = np.asarray(edge_index[0], dtype=np.int64)
    dst = np.asarray(edge_index[1], dtype=np.int64)
    core = dst // NPC
    blk = (dst % NPC) >> 7
    dloc = (dst % NPC) & 127

    def wrap(a):
        n = a.shape[1]
        w = np.ascontiguousarray(a.reshape(NC, n // 16, 16).transpose(0, 2, 1))
        return np.tile(w, (1, 8, 1))  # [NC, 128, n//16]

    def prep_layer(tab, rowid):
        """tab[e] in {0,1}: which gather table; rowid[e]: row within table."""
        key = (core * NB + blk) * 2 + tab
        order = np.lexsort((rowid, key))
        s_row = rowid[order]
        s_dloc = dloc[order]
        s_key = key[order]
        bounds = np.searchsorted(s_key, np.arange(NC * NB * 2 + 1))
        cnt = (bounds[1:] - bounds[:-1]).reshape(NC, NB, 2)
        chunks = _ceil_div(cnt, 128)
        c_lo = chunks[:, :, 0].max(axis=0)
        c_hi = chunks[:, :, 1].max(axis=0)
        nch = c_lo + c_hi
        K_lo = int(c_lo.sum()) * 128
        K_hi = int(c_hi.sum()) * 128
        NCH = int(nch.sum())
        idx_lo = np.zeros((NC, K_lo), np.int16)
        idx_hi = np.zeros((NC, K_hi), np.int16)
        dcols = np.full((NC, NCH * 128), -1.0, np.float32)
        for c in range(NC):
            off_lo = off_hi = off_q = 0
            for b in range(NB):
                i0 = bounds[(c * NB + b) * 2]
                i1 = bounds[(c * NB + b) * 2 + 1]
                i2 = bounds[(c * NB + b) * 2 + 2]
                nlo = i1 - i0
                nhi = i2 - i1
                idx_lo[c, off_lo:off_lo + nlo] = s_row[i0:i1]
                idx_hi[c, off_hi:off_hi + nhi] = s_row[i1:i2]
                dcols[c, off_q:off_q + nlo] = s_dloc[i0:i1]
                dcols[c, off_q + c_lo[b] * 128:off_q + c_lo[b] * 128 + nhi] = s_dloc[i1:i2]
                off_lo += c_lo[b] * 128
                off_hi += c_hi[b] * 128
                off_q += nch[b] * 128
        dl = np.ascontiguousarray(
            dcols.reshape(NC, NCH, 128).transpose(0, 2, 1))
        return dict(c_lo=tuple(int(v) for v in c_lo), c_hi=tuple(int(v) for v in c_hi),
                    idx_lo=wrap(idx_lo), idx_hi=wrap(idx_hi), dl=dl,
                    K_lo=K_lo, K_hi=K_hi, NCH=NCH)

    # layer 1: split x table at row TAB1
    t1 = (src >= TAB1).astype(np.int64)
    r1 = np.where(t1 == 0, src, src - TAB1)
    L1 = prep_layer(t1, r1)
    # layer 2: h is all-gathered in two halves by within-core row ranges
    # (rows < HA_ROWS go to h_full_a, rest to h_full_b)
    rc = src % NPC
    cc = src // NPC
    t2 = (rc >= HA_ROWS).astype(np.int64)
    r2 = np.where(t2 == 0, cc * HA_ROWS + rc, cc * HB_ROWS + (rc - HA_ROWS))
    L2 = prep_layer(t2, r2)

    # per-core 1/max(deg,1), broadcast across partitions (one [128, v] tile
    # per dst block, fused into the onehot build as the mean scaling)
    deg = np.zeros((NC, NPCP), np.float32)
    for c in range(NC):
        deg[c, :NPC] = np.bincount(dst[core == c] % NPC, minlength=NPC)
    recip = (1.0 / np.maximum(deg, 1.0)).astype(bf16)
    recip_bc = np.ascontiguousarray(
        np.broadcast_to(recip[:, None, :], (NC, 128, NPCP)))

    return {
        "L1": L1, "L2": L2, "recip_bc": recip_bc,
        "dstloc": np.concatenate([L1["dl"], L2["dl"]], axis=2),
        "NCH": L1["NCH"] + L2["NCH"],
    }


def _build(L1, L2, NCH, loop_reps=0, stage="full"):
    from concourse import bacc, tile
    from concourse.bass import mybir

    f32 = mybir.dt.float32
    bf16 = mybir.dt.bfloat16
    i16 = mybir.dt.int16
    AF = mybir.ActivationFunctionType
    OP = mybir.AluOpType

    nc = bacc.Bacc("TRN2", target_bir_lowering=False, debug=False, num_devices=NC,
                   num_swdge_queues=4)

    x_tab = nc.declare_dram_parameter("x_tab", [N, F], bf16, isOutput=False)
    xT_self = nc.declare_dram_parameter("xT_self", [128, NPCP], bf16, isOutput=False)
    d_idx1_lo = nc.declare_dram_parameter("idx1_lo", [128, L1["K_lo"] // 16], i16, isOutput=False)
    d_idx1_hi = nc.declare_dram_parameter("idx1_hi", [128, L1["K_hi"] // 16], i16, isOutput=False)
    d_idx2_lo = nc.declare_dram_parameter("idx2_lo", [128, L2["K_lo"] // 16], i16, isOutput=False)
    d_idx2_hi = nc.declare_dram_parameter("idx2_hi", [128, L2["K_hi"] // 16], i16, isOutput=False)
    d_dstloc = nc.declare_dram_parameter("dstloc", [128, NCH], f32, isOutput=False)
    d_recip = nc.declare_dram_parameter("recip_bc", [128, NPCP], bf16, isOutput=False)
    d_wl1 = nc.declare_dram_parameter("wl1", [F, F], bf16, isOutput=False)
    d_wr1 = nc.declare_dram_parameter("wr1", [F, F], bf16, isOutput=False)
    d_wl2 = nc.declare_dram_parameter("wl2", [F, F], bf16, isOutput=False)
    d_wr2 = nc.declare_dram_parameter("wr2", [F, F], bf16, isOutput=False)
    d_b1 = nc.declare_dram_parameter("b1c", [128, 1], f32, isOutput=False)
    d_b2 = nc.declare_dram_parameter("b2c", [128, 1], f32, isOutput=False)
    d_iota = nc.declare_dram_parameter("iota", [128, 128], bf16, isOutput=False)
    d_ident = nc.declare_dram_parameter("ident", [128, 128], bf16, isOutput=False)
    d_outT = nc.declare_dram_parameter("outT", [128, NPCP], f32, isOutput=True)

    # h shard in two pieces so the first AllGather can launch while the
    # second half of layer 1 is still computing, and layer 2's a-side
    # gathers can proceed while the second AllGather is in flight.
    # bf16 payload declared as f32 with wide 2KB rows: bf16 collectives hit
    # a slow path (~1.5x byte-equivalent f32) and collectives run ~2.5x
    # faster with wide rows. Viewed as [rows, 128] bf16 via bitcast/reshape
    # at the gather/store boundaries.
    h_shard_a = nc.dram_tensor("h_shard_a", [HA_ROWS // 8, 512], f32)
    h_shard_b = nc.dram_tensor("h_shard_b", [HB_ROWS // 8, 512], f32)
    h_full_a = nc.dram_tensor("h_full_a", [NC * HA_ROWS // 8, 512], f32, addr_space="Shared")
    h_full_b = nc.dram_tensor("h_full_b", [NC * HB_ROWS // 8, 512], f32, addr_space="Shared")
    h_shard_av = h_shard_a.bitcast(bf16).reshape([HA_ROWS, F])
    h_shard_bv = h_shard_b.bitcast(bf16).reshape([HB_ROWS, F])
    h_full_av = h_full_a.bitcast(bf16).reshape([NC * HA_ROWS, F])
    h_full_bv = h_full_b.bitcast(bf16).reshape([NC * HB_ROWS, F])

    with tile.TileContext(nc) as tc, ExitStack() as ctx:
        pstat = ctx.enter_context(tc.tile_pool(name="stat", bufs=1))
        pg = ctx.enter_context(tc.tile_pool(name="pg", bufs=2))
        pone = ctx.enter_context(tc.tile_pool(name="pone", bufs=6))
        psm = ctx.enter_context(tc.tile_pool(name="psm", bufs=3))
        pnode = ctx.enter_context(tc.tile_pool(name="pnode", bufs=3))
        pps_agg = ctx.enter_context(tc.tile_pool(name="ppsagg", bufs=4, space="PSUM"))
        pps_t = ctx.enter_context(tc.tile_pool(name="ppst", bufs=2, space="PSUM"))
        pps_h = ctx.enter_context(tc.tile_pool(name="ppsh", bufs=2, space="PSUM"))

        iota_s = pstat.tile([128, 128], bf16, tag="iota")
        nc.sync.dma_start(out=iota_s[:], in_=d_iota[:])
        ident_s = pstat.tile([128, 128], bf16, tag="ident")
        nc.sync.dma_start(out=ident_s[:], in_=d_ident[:])
        wl1_s = pstat.tile([128, 128], bf16, tag="wl1")
        nc.sync.dma_start(out=wl1_s[:], in_=d_wl1[:])
        wr1_s = pstat.tile([128, 128], bf16, tag="wr1")
        nc.sync.dma_start(out=wr1_s[:], in_=d_wr1[:])
        wl2_s = pstat.tile([128, 128], bf16, tag="wl2")
        nc.sync.dma_start(out=wl2_s[:], in_=d_wl2[:])
        wr2_s = pstat.tile([128, 128], bf16, tag="wr2")
        nc.sync.dma_start(out=wr2_s[:], in_=d_wr2[:])
        b1_s = pstat.tile([128, 1], f32, tag="b1")
        nc.sync.dma_start(out=b1_s[:], in_=d_b1[:])
        b2_s = pstat.tile([128, 1], f32, tag="b2")
        nc.sync.dma_start(out=b2_s[:], in_=d_b2[:])
        dstloc_s = pstat.tile([128, NCH], f32, tag="dstloc")
        nc.sync.dma_start(out=dstloc_s[:], in_=d_dstloc[:])
        recip_s = pstat.tile([128, NPCP], bf16, tag="recip")
        nc.sync.dma_start(out=recip_s[:], in_=d_recip[:])
        xT_all = pstat.tile([128, NPCP], bf16, tag="xT")
        nc.sync.dma_start(out=xT_all[:], in_=xT_self[:])
        hT_all = pstat.tile([128, NPCP], bf16, tag="hT")
        idx_s = {}
        for nm, d_t, wid in (("1lo", d_idx1_lo, L1["K_lo"]), ("1hi", d_idx1_hi, L1["K_hi"]),
                             ("2lo", d_idx2_lo, L2["K_lo"]), ("2hi", d_idx2_hi, L2["K_hi"])):
            t = pstat.tile([128, wid // 16], i16, tag=f"idx{nm}")
            nc.sync.dma_start(out=t[:], in_=d_t[:])
            idx_s[nm] = t

        self_qn = [0]

        def emit_body(do_ag=True):
            layers = (1, 2)
            if stage == "gather1":
                layers = (1,)
            elif stage == "gather2":
                layers = (2,)
            for layer in layers:
                if layer == 1:
                    LP = L1
                    lo_ap = x_tab[0:TAB1, :]
                    hi_ap = x_tab[TAB1:N, :]
                    t_lo, t_hi = idx_s["1lo"], idx_s["1hi"]
                    wl_s, wr_s, bias_s = wl1_s, wr1_s, b1_s
                    q = 0
                else:
                    LP = L2
                    lo_ap = h_full_av[:]
                    hi_ap = h_full_bv[:]
                    t_lo, t_hi = idx_s["2lo"], idx_s["2hi"]
                    wl_s, wr_s, bias_s = wl2_s, wr2_s, b2_s
                    q = L1["NCH"]
                c_lo, c_hi = LP["c_lo"], LP["c_hi"]
                nch = [c_lo[b] + c_hi[b] for b in range(NB)]
                sbs = [list(range(s, min(s + SBS, NB))) for s in range(0, NB, SBS)]

                off_lo = 0
                off_hi = 0
                for sb in sbs:
                    nlo = sum(c_lo[b] for b in sb) * 128
                    nhi = sum(c_hi[b] for b in sb) * 128
                    skip_gather = stage == "aggonly"

                    # layer 2: keep a-side (h_full_a) gathers on queues {0,1}
                    # and b-side on {2,3} so b-gathers blocked on the second
                    # AllGather don't head-of-line-stall later a-gathers
                    # whose data is already available.
                    def qpick(side):
                        self_qn[0] += 1
                        if layer == 1:
                            return self_qn[0] % 4
                        return (self_qn[0] % 2) + (0 if side == 0 else 2)
                    g_lo = pg.tile([128, nlo // 128, 128], bf16, tag="glo")
                    for o in [] if skip_gather else range(0, nlo, GMAX):
                        nn = min(GMAX, nlo - o)
                        nc.gpsimd.dma_gather(
                            out_ap=g_lo[:, o // 128:(o + nn) // 128, :], in_ap=lo_ap,
                            idxs_ap=t_lo[:, (off_lo + o) // 16:(off_lo + o + nn) // 16],
                            num_idxs=nn, num_idxs_reg=nn, elem_size=F,
                            single_packet=False, queue_num=qpick(0))
                    g_hi = pg.tile([128, nhi // 128, 128], bf16, tag="ghi")
                    for o in [] if skip_gather else range(0, nhi, GMAX):
                        nn = min(GMAX, nhi - o)
                        nc.gpsimd.dma_gather(
                            out_ap=g_hi[:, o // 128:(o + nn) // 128, :], in_ap=hi_ap,
                            idxs_ap=t_hi[:, (off_hi + o) // 16:(off_hi + o + nn) // 16],
                            num_idxs=nn, num_idxs_reg=nn, elem_size=F,
                            single_packet=False, queue_num=qpick(1))
                    off_lo += nlo
                    off_hi += nhi

                    if stage in ("gather", "gather1", "gather2"):
                        q += sum(nch[b] for b in sb)
                        continue
                    col_lo = 0
                    col_hi = 0
                    ps_blocks = {}
                    for b in sb:
                        # all SBS blocks' aggregation matmuls are emitted
                        # before any tail so the in-order PE queue doesn't
                        # stall the next block's aggregation behind ACT
                        # tail work
                        ps_aggT = pps_agg.tile([128, 128], f32, tag="psagg")
                        ps_blocks[b] = ps_aggT
                        for j in range(nch[b]):
                            onehot = pone.tile([128, 128], bf16, tag="onehot")
                            nc.vector.tensor_scalar(
                                onehot[:], iota_s[:], dstloc_s[:, q:q + 1], None,
                                OP.is_equal)
                            if j < c_lo[b]:
                                rhs = g_lo[:, col_lo, :]
                                col_lo += 1
                            else:
                                rhs = g_hi[:, col_hi, :]
                                col_hi += 1
                            nc.tensor.matmul(
                                ps_aggT[:], rhs, onehot[:],
                                start=(j == 0), stop=(j == nch[b] - 1))
                            q += 1
                    if stage == "agg":
                        continue
                    for b in sb:
                        ps_aggT = ps_blocks[b]
                        aggTs = psm.tile([128, 128], bf16, tag="aggTs")
                        nc.vector.scalar_tensor_tensor(
                            aggTs[:], ps_aggT[:], 1.0,
                            recip_s[:, b * 128:(b + 1) * 128], OP.mult, OP.mult)
                        rhs2 = (xT_all if layer == 1 else hT_all)[:, b * 128:(b + 1) * 128]
                        ps_h = pps_h.tile([128, 128], f32, tag="psh")
                        nc.tensor.matmul(ps_h[:], wl_s[:], aggTs[:], start=True, stop=False)
                        nc.tensor.matmul(ps_h[:], wr_s[:], rhs2, start=False, stop=True)
                        if layer == 1:
                            hT_blk = hT_all[:, b * 128:(b + 1) * 128]
                            nc.scalar.activation(hT_blk, ps_h[:], AF.Relu, bias=b1_s[:])
                            ps_t = pps_t.tile([128, 128], bf16, tag="pst")
                            nc.tensor.transpose(ps_t[:], hT_blk, ident_s[:])
                            nodeb = pnode.tile([128, 128], bf16, tag="nodeb")
                            nc.scalar.copy(nodeb[:], ps_t[:])
                            if b < NBA:
                                nc.sync.dma_start(
                                    out=h_shard_av[b * 128:(b + 1) * 128, :],
                                    in_=nodeb[:])
                            else:
                                nc.sync.dma_start(
                                    out=h_shard_bv[(b - NBA) * 128:(b - NBA + 1) * 128, :],
                                    in_=nodeb[:])
                        else:
                            outb = pnode.tile([128, 128], f32, tag="outb")
                            nc.scalar.activation(outb[:], ps_h[:], AF.Relu, bias=b2_s[:])
                            nc.sync.dma_start(
                                out=d_outT[:, b * 128:(b + 1) * 128], in_=outb[:])

                if layer == 1 and do_ag:
                    nc.gpsimd.collective_compute(
                        "AllGather", OP.bypass, replica_groups=[list(range(NC))],
                        ins=[h_shard_a[:]], outs=[h_full_a[:]])
                    nc.gpsimd.collective_compute(
                        "AllGather", OP.bypass, replica_groups=[list(range(NC))],
                        ins=[h_shard_b[:]], outs=[h_full_b[:]])

        if loop_reps:
            # timing variant: collectives can't live inside control flow;
            # init the h_full halves once and loop the 2-layer pipeline
            nc.gpsimd.collective_compute(
                "AllGather", OP.bypass, replica_groups=[list(range(NC))],
                ins=[h_shard_a[:]], outs=[h_full_a[:]])
            nc.gpsimd.collective_compute(
                "AllGather", OP.bypass, replica_groups=[list(range(NC))],
                ins=[h_shard_b[:]], outs=[h_full_b[:]])
            with tc.For_i(0, loop_reps, 1):
                emit_body(do_ag=False)
        else:
            emit_body(do_ag=True)
    nc.compile()
    return nc


def _get_program(prep, loop_reps=0, stage="full"):
    key = (prep["L1"]["c_lo"], prep["L1"]["c_hi"],
           prep["L2"]["c_lo"], prep["L2"]["c_hi"], loop_reps, stage)
    if key not in _cache:
        _cache[key] = _build(prep["L1"], prep["L2"], prep["NCH"], loop_reps, stage)
    return _cache[key]


def _in_maps(prep, x, W1_l, b1, W1_r, W2_l, b2, W2_r):
    import ml_dtypes

    bf16 = ml_dtypes.bfloat16
    x_bf = np.ascontiguousarray(np.asarray(x, np.float32)).astype(bf16)
    iota = np.ascontiguousarray(
        np.broadcast_to(np.arange(128, dtype=np.float32), (128, 128))).astype(bf16)
    ident = np.eye(128, dtype=np.float32).astype(bf16)
    common = {
        "x_tab": x_bf,
        "wl1": np.ascontiguousarray(np.asarray(W1_l, np.float32)).astype(bf16),
        "wr1": np.ascontiguousarray(np.asarray(W1_r, np.float32)).astype(bf16),
        "wl2": np.ascontiguousarray(np.asarray(W2_l, np.float32)).astype(bf16),
        "wr2": np.ascontiguousarray(np.asarray(W2_r, np.float32)).astype(bf16),
        "b1c": np.ascontiguousarray(np.asarray(b1, np.float32).reshape(128, 1)),
        "b2c": np.ascontiguousarray(np.asarray(b2, np.float32).reshape(128, 1)),
        "iota": iota,
        "ident": ident,
    }
    maps = []
    for c in range(NC):
        xTs = np.zeros((128, NPCP), np.float32)
        xTs[:, :NPC] = np.asarray(x, np.float32)[c * NPC:(c + 1) * NPC].T
        m = dict(common)
        m["xT_self"] = np.ascontiguousarray(xTs).astype(bf16)
        m["idx1_lo"] = np.ascontiguousarray(prep["L1"]["idx_lo"][c])
        m["idx1_hi"] = np.ascontiguousarray(prep["L1"]["idx_hi"][c])
        m["idx2_lo"] = np.ascontiguousarray(prep["L2"]["idx_lo"][c])
        m["idx2_hi"] = np.ascontiguousarray(prep["L2"]["idx_hi"][c])
        m["dstloc"] = np.ascontiguousarray(prep["dstloc"][c])
        m["recip_bc"] = np.ascontiguousarray(prep["recip_bc"][c])
        maps.append(m)
    return maps


def kernel(x, edge_index, W1_l, b1, W1_r, W2_l, b2, W2_r):
    from concourse.bass_utils import run_bass_kernel_spmd

    x = np.asarray(x, np.float32)
    assert x.shape == (N, F) and np.asarray(edge_index).shape == (2, E)
    prep = _host_prep(x, edge_index)
    nc = _get_program(prep)
    maps = _in_maps(prep, x, W1_l, b1, W1_r, W2_l, b2, W2_r)
    res = run_bass_kernel_spmd(nc, maps, list(range(NC)))
    out = np.concatenate(
        [np.asarray(res.results[c]["outT"], np.float32).T[:NPC] for c in range(NC)],
        axis=0)
    return out.astype(np.float32)
